# revision 27
# baseline (speedup 1.0000x reference)
"""LocationAwareAttention Trainium2 kernel.

Data-parallel over batch: 16 batch elements / 8 cores = 2 per core.
Each core computes, for its 2 batch elements b:
    conv_feat = conv1d(prev_attn) ; lp = conv_feat @ loc_w.T
    qp = query @ q_w.T ; vp = value @ v_w.T
    e  = tanh(qp + vp + lp + bias)              (computed transposed: d on partitions)
    score = e @ score_w.T + score_b ; sg = sigmoid(score)
    attn = sg / sum(sg) ; context = attn @ value
    out = [context | query] @ out_w.T + out_b

Device layout: value is shipped host-transposed/packed (h-major, chunked) so
the dominant matmul (vp) streams it directly as the PE moving operand from
plain contiguous DMAs; the context reduction (contraction over v, which PE
cannot do in this layout) runs on the vector engine as fused
scalar_tensor_tensor ops with accum_out.  All matmul operands are cast
fp32->bf16 inline by the SWDGE DMA engines.

v chunks are processed in pairs so each PE stationary (LDWEIGHTS) serves two
matmuls, and a warm-up matmul burst trips the PE HAM clock gate to 2.4 GHz
before the main stream begins.
"""

import numpy as np
from contextlib import ExitStack

B, VL, H, D, CO = 16, 4096, 1024, 512, 10
NCORES = 8
BPC = B // NCORES          # batches per core = 2
VCH = 512                  # v chunk size
NCHUNK = VL // VCH         # 8
NKT = H // 128             # 8 k-tiles over hidden
NDT = D // 128             # 4 m-tiles over dim
NOK = (2 * H) // 128       # 16 k-tiles over 2*hidden (out proj)

_CACHE = {}


def _build_program():
    import concourse.bass as bass
    import concourse.tile as tile
    from concourse import bacc, mybir
    from concourse.bass import ds
    from concourse.tile_rust import add_dep_helper

    f32 = mybir.dt.float32
    bf16 = mybir.dt.bfloat16
    AF = mybir.ActivationFunctionType
    AL = mybir.AluOpType

    nc = bacc.Bacc(None, target_bir_lowering=False, debug=False,
                   num_devices=NCORES)

    # ---- DRAM I/O (all pre-packed host-side; device DMAs are plain 2D) ---
    value_p = nc.dram_tensor("value_p", [BPC, NCHUNK, 128, NKT * VCH], f32,
                             kind="ExternalInput")
    prevX = nc.dram_tensor("prevX", [BPC, 4, VL], f32, kind="ExternalInput")
    # packed: [vw 4096 | qw 4096 | swT 4 | q_p 16 | locw 512 | waug 10]
    BLOBW = NKT * D * 2 + NDT + NKT * BPC + D + CO
    blob_p = nc.dram_tensor("blob_p", [128, BLOBW], f32, kind="ExternalInput")
    bias_r = nc.dram_tensor("bias_r", [128, NDT], f32, kind="ExternalInput")
    score_b = nc.dram_tensor("score_b", [1, 1], f32, kind="ExternalInput")
    ow_p = nc.dram_tensor("ow_p", [128, NOK * H], f32, kind="ExternalInput")
    out_b2 = nc.dram_tensor("out_b2", [BPC, H], f32, kind="ExternalInput")

    out_d = nc.dram_tensor("out", [BPC, H], f32, kind="ExternalOutput")
    attn_d = nc.dram_tensor("attn", [BPC, VL], f32, kind="ExternalOutput")
    dbg_d = nc.dram_tensor("dbg", [1, 8], f32, kind="ExternalOutput")

    with tile.TileContext(nc) as tc, ExitStack() as ctx:
        singles = ctx.enter_context(tc.tile_pool(name="singles", bufs=1))
        vt_pool = ctx.enter_context(tc.tile_pool(name="vt", bufs=4))
        te_pool = ctx.enter_context(tc.tile_pool(name="te", bufs=6))
        sgb_pool = ctx.enter_context(tc.tile_pool(name="sgb", bufs=2))
        small = ctx.enter_context(tc.tile_pool(name="small", bufs=2))
        stg_pool = ctx.enter_context(tc.tile_pool(name="stg", bufs=1))
        p_e = ctx.enter_context(tc.tile_pool(name="p_e", bufs=3, space="PSUM"))
        p_sc = ctx.enter_context(tc.tile_pool(name="p_sc", bufs=2, space="PSUM"))
        p_sgb = ctx.enter_context(tc.tile_pool(name="p_sgb", bufs=1, space="PSUM"))
        p_misc = ctx.enter_context(tc.tile_pool(name="p_misc", bufs=2, space="PSUM"))

        # ---- critical-path loads first, serialized so the earliest-needed
        # transfer gets full DMA bandwidth (concurrent SWDGE queues are
        # drained round-robin, which would finish everything late together)
        def cdma(out, in_, gate=None):
            dd = nc.gpsimd.dma_start(out=out, in_=in_)
            if gate is not None:
                add_dep_helper(dd.ins, gate.ins, reason="dma start gate")
            return dd

        vt_sbs = {}
        blob_sb = singles.tile([128, BLOBW], bf16)
        # views into the packed weights blob: [vw | locw | waug | swT | q_p | qw]
        OFF_VW = 0
        OFF_LW = NKT * D
        OFF_WA = OFF_LW + D
        OFF_SW = OFF_WA + CO
        OFF_QP = OFF_SW + NDT
        OFF_QW = OFF_QP + NKT * BPC
        BLOBA = OFF_QW
        RESTA = BLOBA - NKT * D   # locw/waug/swT/q_p tail of part A
        qsz = NKT * 128

        # Startup-critical tensors ride the two HWDGE rings as plain fp32
        # (full DMA rate; the SWDGE cast path tops out around half rate for
        # a single in-flight transfer) and are cast to bf16 on the
        # still-idle vector engine.
        HS = NKT * VCH // 2
        vt0 = vt_pool.tile([128, NKT * VCH], bf16, tag="vt", name="vt00")
        for h in range(2):
            stg = stg_pool.tile([128, HS], f32, tag="stg", name=f"stg_a{h}")
            nc.sync.dma_start(out=stg, in_=value_p[0, 0][:, ds(h * HS, HS)])
            nc.vector.tensor_copy(vt0[:, ds(h * HS, HS)], stg)
        for h in range(2):
            stg = stg_pool.tile([128, HS], f32, tag="stg2", name=f"stg_b{h}")
            nc.scalar.dma_start(out=stg, in_=blob_p[:, ds(h * HS, HS)])
            nc.vector.tensor_copy(blob_sb[:, ds(h * HS, HS)], stg)
        px0_sb = small.tile([4, VL], bf16, tag="px", name="px0")
        cdma(px0_sb, prevX[0])
        stg_c = stg_pool.tile([128, HS], f32, tag="stg2", name="stg_c")
        nc.scalar.dma_start(out=stg_c[:, ds(0, RESTA)],
                            in_=blob_p[:, ds(NKT * D, RESTA)])
        nc.vector.tensor_copy(blob_sb[:, ds(NKT * D, RESTA)],
                              stg_c[:, ds(0, RESTA)])
        stg_d = stg_pool.tile([128, HS], f32, tag="stg", name="stg_d")
        nc.sync.dma_start(out=stg_d[:, ds(0, qsz)],
                          in_=blob_p[:, ds(OFF_QW, qsz)])
        nc.vector.tensor_copy(blob_sb[:, ds(OFF_QW, qsz)],
                              stg_d[:, ds(0, qsz)])
        vw_sb = blob_sb[:, ds(OFF_VW, NKT * D)]
        qw_sb = blob_sb[:, ds(OFF_QW, NKT * D)]
        swT_sb = blob_sb[:, ds(OFF_SW, NDT)]
        locw_sb = blob_sb[0:CO, ds(OFF_LW, D)]
        waug_sb = blob_sb[0:4, ds(OFF_WA, CO)]

        # ---- PE warm-up: dense junk matmuls trip HAM to 2.4 GHz ----------
        warm_sb = singles.tile([128, 128], bf16)
        nc.vector.memset(warm_sb, 0.001)
        warm_ps = p_e.tile([128, 128], f32, tag="e")
        warm_last = None
        for i in range(64):
            warm_last = nc.tensor.matmul(warm_ps, warm_sb, warm_sb,
                                         start=(i == 0), stop=(i == 63))
        dbg_sb = singles.tile([1, 8], f32)
        nc.scalar.activation(dbg_sb[:, 0:4], warm_ps[0:1, 0:4], AF.Copy)

        # second-wave loads start once the warm-up burst retires, leaving
        # the full DMA bandwidth to the group-0 prerequisites before that
        for jq in range(1, NDT):
            cdma(blob_sb[:, ds(OFF_QW + jq * qsz, qsz)],
                 blob_p[:, ds(OFF_QW + jq * qsz, qsz)], gate=warm_last)
        for c in (1, 2):
            vtx = vt_pool.tile([128, NKT * VCH], bf16, tag="vt", name=f"vt0{c}")
            cdma(vtx, value_p[0, c], gate=warm_last)
            vt_sbs[(0, c)] = vtx
        px1_sb = small.tile([4, VL], bf16, tag="px", name="px1")
        cdma(px1_sb, prevX[1], gate=warm_last)
        px_sbs = [px0_sb, px1_sb]

        # ---- remaining resident weights ----------------------------------
        bias_sb = singles.tile([128, NDT], f32)
        nc.sync.dma_start(out=bias_sb, in_=bias_r[:, :])
        sb_sb = singles.tile([1, 1], f32)
        nc.sync.dma_start(out=sb_sb, in_=score_b[:, :])
        outb_sb = singles.tile([BPC, H], f32)
        nc.sync.dma_start(out=outb_sb, in_=out_b2[:, :])

        outw_sb = singles.tile([128, NOK * H], bf16)
        ctxT_sb = singles.tile([128, NKT * BPC], bf16)
        ones_sb = singles.tile([1, 128], bf16)
        nc.vector.memset(ones_sb, 1.0)
        onesf_sb = singles.tile([1, 128], f32)
        nc.vector.memset(onesf_sb, 1.0)

        # per-batch bookkeeping
        sg_sb = singles.tile([1, BPC, VL], f32)
        ssum_sb = singles.tile([1, BPC, NCHUNK], f32)
        ctxcols = singles.tile([128, BPC, NKT, NCHUNK], f32)
        ctxred = singles.tile([128, BPC, NKT], f32)
        recip_sb = singles.tile([1, BPC], f32)
        qpb_sb = singles.tile([128, NDT, BPC], f32)
        junk_sb = singles.tile([128, VCH], bf16)

        # conv-feature chunks are produced inside the main loop (keeps PE
        # dense from the start); qp is emitted after the first group.

        def batch_epilogue(b):
            """context reduce -> combT ctx cols; then attn out.  (ssum
            reduction + reciprocal were already emitted after the batch's
            last sigmoid.)"""
            nc.vector.tensor_reduce(ctxred[:, b, :], ctxcols[:, b, :, :],
                                    axis=mybir.AxisListType.X, op=AL.add)
            # fp32 rank-1 broadcast of 1/S to 128 partitions
            rb_ps = p_sgb.tile([128, 1], f32, tag="sgbp", name=f"rb{b}")
            nc.tensor.matmul(rb_ps, onesf_sb, recip_sb[:, b:b + 1],
                             start=True, stop=True)
            rb_sb = small.tile([128, 1], f32, tag="rbs", name=f"rbs{b}")
            nc.scalar.copy(rb_sb, rb_ps)
            ctxT_v = ctxT_sb.rearrange("p (k b) -> p k b", b=BPC)
            nc.vector.tensor_scalar_mul(ctxT_v[:, 0:NKT, b], ctxred[:, b, :],
                                        rb_sb)
            attn_sb = small.tile([1, VL], f32, tag="attn", name=f"attn{b}",
                                 bufs=1)
            nc.vector.tensor_scalar_mul(attn_sb, sg_sb[:, b, :],
                                        recip_sb[:, b:b + 1])
            nc.sync.dma_start(out=attn_d[b], in_=attn_sb)

        # ---- main loop: chunk groups, shared stationaries ---------------
        GROUPS = {0: [(0,), (1, 2), (3, 4), (5, 6), (7,)],
                  1: [(0, 1), (2, 3), (4, 5), (6,), (7,)]}
        def emit_qp(j):
            qpp = p_misc.tile([128, BPC], f32, tag="misc", name=f"qpp{j}")
            for ki in range(NKT):
                nc.tensor.matmul(
                    qpp, qw_sb[:, ds(j * NKT * 128 + ki * 128, 128)],
                    blob_sb[:, ds(OFF_QP + ki * BPC, BPC)],
                    start=(ki == 0), stop=(ki == NKT - 1))
            nc.vector.tensor_scalar_add(qpb_sb[:, j, :], qpp,
                                        bias_sb[:, j:j + 1])
        cf_sbs = []
        group_mms = []
        gidx = 0
        for b in range(BPC):
            cf_sb = small.tile([CO, VL], bf16, tag="cf", name=f"cf{b}")
            cf_sbs.append(cf_sb)
            for gi, grp in enumerate(GROUPS[b]):
                L = len(grp)
                vts = []
                gate = group_mms[gidx - 2] if gidx >= 2 else None
                for c in grp:
                    if (b, c) in vt_sbs:
                        vts.append(vt_sbs[(b, c)])
                    else:
                        vt = vt_pool.tile([128, NKT * VCH], bf16, tag="vt",
                                          name=f"vt{b}_{c}")
                        cdma(vt, value_p[b, c], gate=gate)
                        vts.append(vt)
                # conv features for this group's chunks
                for c in grp:
                    cfp = p_misc.tile([CO, VCH], f32, tag="misc",
                                      name=f"cfp{b}_{c}")
                    nc.tensor.matmul(cfp, waug_sb, px_sbs[b][:, ds(c * VCH, VCH)],
                                     start=True, stop=True)
                    nc.scalar.activation(cf_sb[:, ds(c * VCH, VCH)], cfp,
                                         AF.Copy)
                sc_ps = [p_sc.tile([1, VCH], f32, tag="sc", name=f"sc{b}_{gi}_{x}")
                         for x in range(L)]
                tes = {}
                for j in range(NDT):
                    e_ps = [p_e.tile([128, VCH], f32, tag="e",
                                     name=f"e{b}_{gi}_{j}_{x}")
                            for x in range(L)]
                    for ki in range(NKT):
                        lhs = vw_sb[:, ds(ki * D + j * 128, 128)]
                        for x in range(L):
                            mm = nc.tensor.matmul(e_ps[x], lhs,
                                                  vts[x][:, ds(ki * VCH, VCH)],
                                                  start=(ki == 0), stop=False)
                            if j == 0 and ki == 0 and x == 0:
                                group_mms.append(mm)
                    lhs = locw_sb[:, ds(j * 128, 128)]
                    for x in range(L):
                        nc.tensor.matmul(e_ps[x], lhs,
                                         cf_sb[:, ds(grp[x] * VCH, VCH)],
                                         start=False, stop=True)
                    if b == 0 and gi == 0:
                        # query projection for this j slots in here: its
                        # j-slice of qw lands while the e-block streams
                        emit_qp(j)
                    for x in range(L):
                        t = te_pool.tile([128, VCH], bf16, tag="te",
                                         name=f"te{b}_{gi}_{j}_{x}")
                        nc.scalar.activation(t, e_ps[x], AF.Tanh,
                                             bias=qpb_sb[:, j, b:b + 1])
                        tes[(j, x)] = t
                    if j > 0:
                        lhs = swT_sb[:, j - 1:j]
                        for x in range(L):
                            nc.tensor.matmul(sc_ps[x], lhs, tes[(j - 1, x)],
                                             start=(j == 1), stop=False)
                jl = NDT - 1
                lhs = swT_sb[:, jl:jl + 1]
                for x in range(L):
                    nc.tensor.matmul(sc_ps[x], lhs, tes[(jl, x)],
                                     start=False, stop=True)
                for x, c in enumerate(grp):
                    # sigmoid + per-chunk sum of sg
                    nc.scalar.activation(sg_sb[:, b, ds(c * VCH, VCH)],
                                         sc_ps[x], AF.Sigmoid,
                                         bias=sb_sb[:, 0:1],
                                         accum_out=ssum_sb[:, b, c:c + 1])
                    if c == NCHUNK - 1:
                        # 1/S ready before the context backlog drains
                        ssr = small.tile([1, 1], f32, tag="ssr",
                                         name=f"ssr{b}")
                        nc.vector.tensor_reduce(ssr, ssum_sb[:, b, :],
                                                axis=mybir.AxisListType.X,
                                                op=AL.add)
                        nc.vector.reciprocal(recip_sb[:, b:b + 1], ssr)
                    # bf16 copy of sg for the broadcast matmul
                    sgc_sb = sgb_pool.tile([1, VCH], bf16, tag="sgc",
                                           name=f"sgc{b}_{c}")
                    nc.scalar.activation(sgc_sb, sg_sb[:, b, ds(c * VCH, VCH)],
                                         AF.Copy)
                    sgb_ps = p_sgb.tile([128, VCH], f32, tag="sgbp",
                                        name=f"sgbp{b}_{c}")
                    nc.tensor.matmul(sgb_ps, ones_sb, sgc_sb,
                                     start=True, stop=True)
                    sgb_sb = sgb_pool.tile([128, VCH], bf16, tag="sgb",
                                           name=f"sgb{b}_{c}")
                    nc.scalar.activation(sgb_sb, sgb_ps, AF.Copy)
                    # context partials: ctxcols[., b, ki, c] = sum_v vt*sg
                    for ki in range(NKT):
                        nc.vector.scalar_tensor_tensor(
                            out=junk_sb, in0=vts[x][:, ds(ki * VCH, VCH)],
                            scalar=0.0, in1=sgb_sb,
                            op0=AL.bypass, op1=AL.mult,
                            accum_out=ctxcols[:, b, ki, c:c + 1])
                if b == 0:
                    ng = len(GROUPS[0])
                    s0 = (gi * NOK * H) // ng
                    s1 = ((gi + 1) * NOK * H) // ng
                    cdma(outw_sb[:, ds(s0, s1 - s0)],
                         ow_p[:, ds(s0, s1 - s0)], gate=group_mms[-1])
                gidx += 1

            if b == 0:
                batch_epilogue(0)

        # query half of the output projection (independent of context)
        op_ps = [p_sc.tile([BPC, 512], f32, tag="sc", name=f"op{x}")
                 for x in range(2)]
        for n in range(2):
            for ki in range(NKT, NOK):
                nc.tensor.matmul(op_ps[n],
                                 blob_sb[:, ds(OFF_QP + (ki - NKT) * BPC, BPC)],
                                 outw_sb[:, ds(ki * H + n * 512, 512)],
                                 start=(ki == NKT), stop=False)

        # keep the PE HAM warm while the batch-1 context backlog drains
        warm2_ps = p_e.tile([128, 128], f32, tag="e", name="warm2")
        for i in range(100):
            nc.tensor.matmul(warm2_ps, warm_sb, warm_sb,
                             start=(i == 0), stop=(i == 99))
        nc.scalar.activation(dbg_sb[:, 4:8], warm2_ps[0:1, 0:4], AF.Copy)
        nc.sync.dma_start(out=dbg_d[:, :], in_=dbg_sb)

        batch_epilogue(1)

        # context half + bias + store
        for n in range(2):
            for ki in range(NKT):
                nc.tensor.matmul(op_ps[n], ctxT_sb[:, ds(ki * BPC, BPC)],
                                 outw_sb[:, ds(ki * H + n * 512, 512)],
                                 start=False, stop=(ki == NKT - 1))
            o_sb = small.tile([BPC, 512], f32, tag="osb", name=f"o{n}")
            nc.vector.tensor_add(o_sb, op_ps[n], outb_sb[:, ds(n * 512, 512)])
            nc.sync.dma_start(out=out_d[:, ds(n * 512, 512)], in_=o_sb)

    nc.compile()
    return nc


def _get_program():
    if "nc" not in _CACHE:
        _CACHE["nc"] = _build_program()
    return _CACHE["nc"]


def _host_prep(query, value, prev_attn, conv_w, conv_b, loc_w, q_w, v_w, bias,
               score_w, score_b, out_w, out_b):
    """Build per-core input maps (layout transforms only)."""
    query = np.asarray(query, np.float32)
    value = np.asarray(value, np.float32)
    prev_attn = np.asarray(prev_attn, np.float32)

    # shifted prev_attn rows + ones row (conv via matmul, bias folded)
    px = np.zeros((B, 4, VL), np.float32)
    px[:, 0, 1:] = prev_attn[:, :-1]
    px[:, 1, :] = prev_attn
    px[:, 2, :-1] = prev_attn[:, 1:]
    px[:, 3, :] = 1.0

    w_aug = np.zeros((4, CO), np.float32)
    w_aug[0:3] = np.asarray(conv_w, np.float32)[:, 0, :].T  # [t, c]
    w_aug[3] = np.asarray(conv_b, np.float32)

    def pack_w(w, nkt):
        # (out_dim, in_dim) weight -> [128, nkt*out_dim] with k-tile-major free
        wt = np.ascontiguousarray(np.asarray(w, np.float32).T)  # (in, out)
        od = wt.shape[1]
        return np.ascontiguousarray(
            wt.reshape(nkt, 128, od).transpose(1, 0, 2).reshape(128, nkt * od))

    locw_pad = np.zeros((128, D), np.float32)
    locw_pad[:CO] = np.asarray(loc_w, np.float32).T
    waug_pad = np.zeros((128, CO), np.float32)
    waug_pad[:4] = w_aug
    score_wR = np.asarray(score_w, np.float32)[0].reshape(NDT, 128).T

    shared = {
        "bias_r": np.ascontiguousarray(
            np.asarray(bias, np.float32).reshape(NDT, 128).T),
        "score_b": np.asarray(score_b, np.float32).reshape(1, 1),
        "ow_p": pack_w(out_w, NOK),
        "out_b2": np.ascontiguousarray(
            np.broadcast_to(np.asarray(out_b, np.float32), (BPC, H))),
    }
    in_maps = []
    for cidx in range(NCORES):
        sl = slice(cidx * BPC, (cidx + 1) * BPC)
        m = dict(shared)
        # value[b, v, h] -> [b, chunk, p, ki*VCH + vv] with h = ki*128 + p,
        # v = chunk*VCH + vv
        vv = value[sl].reshape(BPC, NCHUNK, VCH, NKT, 128)
        m["value_p"] = np.ascontiguousarray(
            vv.transpose(0, 1, 4, 3, 2).reshape(BPC, NCHUNK, 128, NKT * VCH))
        # query[b, 0, h] -> [p, ki*BPC + b]
        qq = query[sl, 0, :].T.reshape(NKT, 128, BPC)
        q_p = np.ascontiguousarray(
            qq.transpose(1, 0, 2).reshape(128, NKT * BPC))
        qwt = np.asarray(q_w, np.float32).T.reshape(NKT, 128, NDT, 128)
        qw_jmaj = qwt.transpose(1, 2, 0, 3).reshape(128, NKT * D)
        m["blob_p"] = np.ascontiguousarray(np.concatenate(
            [pack_w(v_w, NKT), locw_pad, waug_pad, score_wR, q_p,
             qw_jmaj], axis=1))
        m["prevX"] = np.ascontiguousarray(px[sl])
        in_maps.append(m)
    return in_maps


def kernel(query, value, prev_attn, conv_w, conv_b, loc_w, q_w, v_w, bias,
           score_w, score_b, out_w, out_b, seq_len=None, **_unused):
    from concourse.bass_utils import run_bass_kernel_spmd

    nc = _get_program()
    in_maps = _host_prep(query, value, prev_attn, conv_w, conv_b, loc_w,
                         q_w, v_w, bias, score_w, score_b, out_w, out_b)
    res = run_bass_kernel_spmd(nc, in_maps, list(range(NCORES)))
    _CACHE["last_results"] = res
    output = np.zeros((B, 1, H), np.float32)
    attn = np.zeros((B, VL), np.float32)
    for cidx in range(NCORES):
        sl = slice(cidx * BPC, (cidx + 1) * BPC)
        output[sl, 0, :] = res.results[cidx]["out"]
        attn[sl, :] = res.results[cidx]["attn"]
    return output, attn


# revision 28
# speedup vs baseline: 1.2476x; 1.2476x over previous
"""LocationAwareAttention Trainium2 kernel.

Data-parallel over batch: 16 batch elements / 8 cores = 2 per core.
Each core computes, for its 2 batch elements b:
    conv_feat = conv1d(prev_attn) ; lp = conv_feat @ loc_w.T
    qp = query @ q_w.T ; vp = value @ v_w.T
    e  = tanh(qp + vp + lp + bias)              (computed transposed: d on partitions)
    score = e @ score_w.T + score_b ; sg = sigmoid(score)
    attn = sg / sum(sg) ; context = attn @ value
    out = [context | query] @ out_w.T + out_b

Device layout: value is shipped host-transposed/packed (h-major, chunked) so
the dominant matmul (vp) streams it directly as the PE moving operand from
plain contiguous DMAs; the context reduction (contraction over v, which PE
cannot do in this layout) runs on the vector engine as fused
scalar_tensor_tensor ops with accum_out.  All matmul operands are cast
fp32->bf16 inline by the SWDGE DMA engines.

v chunks are processed in pairs so each PE stationary (LDWEIGHTS) serves two
matmuls, and a warm-up matmul burst trips the PE HAM clock gate to 2.4 GHz
before the main stream begins.
"""

import numpy as np
from contextlib import ExitStack

B, VL, H, D, CO = 16, 4096, 1024, 512, 10
NCORES = 8
BPC = B // NCORES          # batches per core = 2
VCH = 512                  # v chunk size
NCHUNK = VL // VCH         # 8
NKT = H // 128             # 8 k-tiles over hidden
NDT = D // 128             # 4 m-tiles over dim
NOK = (2 * H) // 128       # 16 k-tiles over 2*hidden (out proj)

_CACHE = {}


def _build_program():
    import concourse.bass as bass
    import concourse.tile as tile
    from concourse import bacc, mybir
    from concourse.bass import ds
    from concourse.tile_rust import add_dep_helper

    f32 = mybir.dt.float32
    bf16 = mybir.dt.bfloat16
    AF = mybir.ActivationFunctionType
    AL = mybir.AluOpType

    nc = bacc.Bacc(None, target_bir_lowering=False, debug=False,
                   num_devices=NCORES)

    # ---- DRAM I/O (all pre-packed host-side; device DMAs are plain 2D) ---
    value_p = nc.dram_tensor("value_p", [BPC, NCHUNK, 128, NKT * VCH], f32,
                             kind="ExternalInput")
    prevX = nc.dram_tensor("prevX", [BPC, 4, VL], f32, kind="ExternalInput")
    # packed: [vw 4096 | qw 4096 | swT 4 | q_p 16 | locw 512 | waug 10]
    BLOBW = NKT * D * 2 + NDT + NKT * BPC + D + CO
    blob_p = nc.dram_tensor("blob_p", [128, BLOBW], f32, kind="ExternalInput")
    bias_r = nc.dram_tensor("bias_r", [128, NDT], f32, kind="ExternalInput")
    score_b = nc.dram_tensor("score_b", [1, 1], f32, kind="ExternalInput")
    ow_p = nc.dram_tensor("ow_p", [128, NOK * H], f32, kind="ExternalInput")
    out_b2 = nc.dram_tensor("out_b2", [BPC, H], f32, kind="ExternalInput")

    out_d = nc.dram_tensor("out", [BPC, H], f32, kind="ExternalOutput")
    attn_d = nc.dram_tensor("attn", [BPC, VL], f32, kind="ExternalOutput")
    dbg_d = nc.dram_tensor("dbg", [1, 8], f32, kind="ExternalOutput")

    with tile.TileContext(nc) as tc, ExitStack() as ctx:
        singles = ctx.enter_context(tc.tile_pool(name="singles", bufs=1))
        vt_pool = ctx.enter_context(tc.tile_pool(name="vt", bufs=6))
        te_pool = ctx.enter_context(tc.tile_pool(name="te", bufs=6))
        sgb_pool = ctx.enter_context(tc.tile_pool(name="sgb", bufs=2))
        small = ctx.enter_context(tc.tile_pool(name="small", bufs=2))
        p_e = ctx.enter_context(tc.tile_pool(name="p_e", bufs=3, space="PSUM"))
        p_sc = ctx.enter_context(tc.tile_pool(name="p_sc", bufs=2, space="PSUM"))
        p_sgb = ctx.enter_context(tc.tile_pool(name="p_sgb", bufs=1, space="PSUM"))
        p_misc = ctx.enter_context(tc.tile_pool(name="p_misc", bufs=2, space="PSUM"))

        # ---- critical-path loads first, serialized so the earliest-needed
        # transfer gets full DMA bandwidth (concurrent SWDGE queues are
        # drained round-robin, which would finish everything late together)
        def cdma(out, in_, gate=None):
            dd = nc.gpsimd.dma_start(out=out, in_=in_)
            if gate is not None:
                add_dep_helper(dd.ins, gate.ins, reason="dma start gate")
            return dd

        vt_sbs = {}
        blob_sb = singles.tile([128, BLOBW], bf16)
        # views into the packed weights blob: [vw | locw | waug | swT | q_p | qw]
        OFF_VW = 0
        OFF_LW = NKT * D
        OFF_WA = OFF_LW + D
        OFF_SW = OFF_WA + CO
        OFF_QP = OFF_SW + NDT
        OFF_QW = OFF_QP + NKT * BPC
        BLOBA = OFF_QW
        RESTA = BLOBA - NKT * D   # locw/waug/swT/q_p tail of part A
        qsz = NKT * 128

        # Startup-critical tensors ride the two HWDGE rings as plain fp32
        # (full DMA rate; the SWDGE cast path tops out around half rate for
        # a single in-flight transfer) and are cast to bf16 on the
        # still-idle vector engine.
        vt0 = vt_pool.tile([128, NKT * VCH], bf16, tag="vt", name="vt00")
        cdma(vt0, value_p[0, 0])
        cdma(blob_sb[:, ds(0, BLOBA)], blob_p[:, ds(0, BLOBA)])
        px0_sb = small.tile([4, VL], bf16, tag="px", name="px0")
        cdma(px0_sb, prevX[0])
        cdma(blob_sb[:, ds(OFF_QW, qsz)], blob_p[:, ds(OFF_QW, qsz)])
        vw_sb = blob_sb[:, ds(OFF_VW, NKT * D)]
        qw_sb = blob_sb[:, ds(OFF_QW, NKT * D)]
        swT_sb = blob_sb[:, ds(OFF_SW, NDT)]
        locw_sb = blob_sb[0:CO, ds(OFF_LW, D)]
        waug_sb = blob_sb[0:4, ds(OFF_WA, CO)]

        # ---- PE warm-up: dense junk matmuls trip HAM to 2.4 GHz ----------
        warm_sb = singles.tile([128, 128], bf16)
        nc.vector.memset(warm_sb, 0.001)
        warm_ps = p_e.tile([128, 128], f32, tag="e")
        warm_last = None
        for i in range(190):
            warm_last = nc.tensor.matmul(warm_ps, warm_sb, warm_sb,
                                         start=(i == 0), stop=(i == 189))
        dbg_sb = singles.tile([1, 8], f32)
        nc.scalar.activation(dbg_sb[:, 0:4], warm_ps[0:1, 0:4], AF.Copy)

        # second-wave loads start once the warm-up burst retires, leaving
        # the full DMA bandwidth to the group-0 prerequisites before that
        for jq in range(1, NDT):
            cdma(blob_sb[:, ds(OFF_QW + jq * qsz, qsz)],
                 blob_p[:, ds(OFF_QW + jq * qsz, qsz)], gate=warm_last)
        for c in (1, 2):
            vtx = vt_pool.tile([128, NKT * VCH], bf16, tag="vt", name=f"vt0{c}")
            cdma(vtx, value_p[0, c], gate=warm_last)
            vt_sbs[(0, c)] = vtx
        px1_sb = small.tile([4, VL], bf16, tag="px", name="px1")
        cdma(px1_sb, prevX[1], gate=warm_last)
        px_sbs = [px0_sb, px1_sb]

        # ---- remaining resident weights ----------------------------------
        bias_sb = singles.tile([128, NDT], f32)
        nc.sync.dma_start(out=bias_sb, in_=bias_r[:, :])
        sb_sb = singles.tile([1, 1], f32)
        nc.sync.dma_start(out=sb_sb, in_=score_b[:, :])
        outb_sb = singles.tile([BPC, H], f32)
        nc.sync.dma_start(out=outb_sb, in_=out_b2[:, :])

        outw_sb = singles.tile([128, NOK * H], bf16)
        ctxT_sb = singles.tile([128, NKT * BPC], bf16)
        ones_sb = singles.tile([1, 128], bf16)
        nc.vector.memset(ones_sb, 1.0)
        onesf_sb = singles.tile([1, 128], f32)
        nc.vector.memset(onesf_sb, 1.0)

        # per-batch bookkeeping
        sg_sb = singles.tile([1, BPC, VL], f32)
        ssum_sb = singles.tile([1, BPC, NCHUNK], f32)
        ctxcols = singles.tile([128, BPC, NKT, NCHUNK], f32)
        ctxred = singles.tile([128, BPC, NKT], f32)
        recip_sb = singles.tile([1, BPC], f32)
        qpb_sb = singles.tile([128, NDT, BPC], f32)
        junk_sb = singles.tile([128, VCH], bf16)

        # conv-feature chunks are produced inside the main loop (keeps PE
        # dense from the start); qp is emitted after the first group.

        def batch_epilogue(b):
            """context reduce -> combT ctx cols; then attn out.  (ssum
            reduction + reciprocal were already emitted after the batch's
            last sigmoid.)"""
            nc.vector.tensor_reduce(ctxred[:, b, :], ctxcols[:, b, :, :],
                                    axis=mybir.AxisListType.X, op=AL.add)
            # fp32 rank-1 broadcast of 1/S to 128 partitions
            rb_ps = p_sgb.tile([128, 1], f32, tag="sgbp", name=f"rb{b}")
            nc.tensor.matmul(rb_ps, onesf_sb, recip_sb[:, b:b + 1],
                             start=True, stop=True)
            rb_sb = small.tile([128, 1], f32, tag="rbs", name=f"rbs{b}")
            nc.scalar.copy(rb_sb, rb_ps)
            ctxT_v = ctxT_sb.rearrange("p (k b) -> p k b", b=BPC)
            nc.vector.tensor_scalar_mul(ctxT_v[:, 0:NKT, b], ctxred[:, b, :],
                                        rb_sb)
            attn_sb = small.tile([1, VL], f32, tag="attn", name=f"attn{b}",
                                 bufs=1)
            nc.vector.tensor_scalar_mul(attn_sb, sg_sb[:, b, :],
                                        recip_sb[:, b:b + 1])
            nc.sync.dma_start(out=attn_d[b], in_=attn_sb)

        # ---- main loop: chunk groups, shared stationaries ---------------
        GROUPS = {0: [(0,), (1, 2), (3, 4), (5, 6), (7,)],
                  1: [(0, 1), (2, 3), (4, 5), (6,), (7,)]}
        def emit_qp(j):
            qpp = p_misc.tile([128, BPC], f32, tag="misc", name=f"qpp{j}")
            for ki in range(NKT):
                nc.tensor.matmul(
                    qpp, qw_sb[:, ds(j * NKT * 128 + ki * 128, 128)],
                    blob_sb[:, ds(OFF_QP + ki * BPC, BPC)],
                    start=(ki == 0), stop=(ki == NKT - 1))
            nc.vector.tensor_scalar_add(qpb_sb[:, j, :], qpp,
                                        bias_sb[:, j:j + 1])
        cf_sbs = []
        group_mms = []
        gidx = 0
        for b in range(BPC):
            cf_sb = small.tile([CO, VL], bf16, tag="cf", name=f"cf{b}")
            cf_sbs.append(cf_sb)
            for gi, grp in enumerate(GROUPS[b]):
                L = len(grp)
                vts = []
                gate = group_mms[gidx - 2] if gidx >= 2 else None
                for c in grp:
                    if (b, c) in vt_sbs:
                        vts.append(vt_sbs[(b, c)])
                    else:
                        vt = vt_pool.tile([128, NKT * VCH], bf16, tag="vt",
                                          name=f"vt{b}_{c}")
                        cdma(vt, value_p[b, c], gate=gate)
                        vts.append(vt)
                # conv features for this group's chunks
                for c in grp:
                    cfp = p_misc.tile([CO, VCH], f32, tag="misc",
                                      name=f"cfp{b}_{c}")
                    nc.tensor.matmul(cfp, waug_sb, px_sbs[b][:, ds(c * VCH, VCH)],
                                     start=True, stop=True)
                    nc.scalar.activation(cf_sb[:, ds(c * VCH, VCH)], cfp,
                                         AF.Copy)
                sc_ps = [p_sc.tile([1, VCH], f32, tag="sc", name=f"sc{b}_{gi}_{x}")
                         for x in range(L)]
                tes = {}
                for j in range(NDT):
                    e_ps = [p_e.tile([128, VCH], f32, tag="e",
                                     name=f"e{b}_{gi}_{j}_{x}")
                            for x in range(L)]
                    for ki in range(NKT):
                        lhs = vw_sb[:, ds(ki * D + j * 128, 128)]
                        for x in range(L):
                            mm = nc.tensor.matmul(e_ps[x], lhs,
                                                  vts[x][:, ds(ki * VCH, VCH)],
                                                  start=(ki == 0), stop=False)
                            if j == 0 and ki == 0 and x == 0:
                                group_mms.append(mm)
                    lhs = locw_sb[:, ds(j * 128, 128)]
                    for x in range(L):
                        nc.tensor.matmul(e_ps[x], lhs,
                                         cf_sb[:, ds(grp[x] * VCH, VCH)],
                                         start=False, stop=True)
                    if b == 0 and gi == 0:
                        # query projection for this j slots in here: its
                        # j-slice of qw lands while the e-block streams
                        emit_qp(j)
                    for x in range(L):
                        t = te_pool.tile([128, VCH], bf16, tag="te",
                                         name=f"te{b}_{gi}_{j}_{x}")
                        nc.scalar.activation(t, e_ps[x], AF.Tanh,
                                             bias=qpb_sb[:, j, b:b + 1])
                        tes[(j, x)] = t
                    if j > 0:
                        lhs = swT_sb[:, j - 1:j]
                        for x in range(L):
                            nc.tensor.matmul(sc_ps[x], lhs, tes[(j - 1, x)],
                                             start=(j == 1), stop=False)
                jl = NDT - 1
                lhs = swT_sb[:, jl:jl + 1]
                for x in range(L):
                    nc.tensor.matmul(sc_ps[x], lhs, tes[(jl, x)],
                                     start=False, stop=True)
                for x, c in enumerate(grp):
                    # sigmoid + per-chunk sum of sg
                    nc.scalar.activation(sg_sb[:, b, ds(c * VCH, VCH)],
                                         sc_ps[x], AF.Sigmoid,
                                         bias=sb_sb[:, 0:1],
                                         accum_out=ssum_sb[:, b, c:c + 1])
                    if c == NCHUNK - 1:
                        # 1/S ready before the context backlog drains
                        ssr = small.tile([1, 1], f32, tag="ssr",
                                         name=f"ssr{b}")
                        nc.vector.tensor_reduce(ssr, ssum_sb[:, b, :],
                                                axis=mybir.AxisListType.X,
                                                op=AL.add)
                        nc.vector.reciprocal(recip_sb[:, b:b + 1], ssr)
                    # bf16 copy of sg for the broadcast matmul
                    sgc_sb = sgb_pool.tile([1, VCH], bf16, tag="sgc",
                                           name=f"sgc{b}_{c}")
                    nc.scalar.activation(sgc_sb, sg_sb[:, b, ds(c * VCH, VCH)],
                                         AF.Copy)
                    sgb_ps = p_sgb.tile([128, VCH], f32, tag="sgbp",
                                        name=f"sgbp{b}_{c}")
                    nc.tensor.matmul(sgb_ps, ones_sb, sgc_sb,
                                     start=True, stop=True)
                    sgb_sb = sgb_pool.tile([128, VCH], bf16, tag="sgb",
                                           name=f"sgb{b}_{c}")
                    nc.scalar.activation(sgb_sb, sgb_ps, AF.Copy)
                    # context partials: ctxcols[., b, ki, c] = sum_v vt*sg
                    for ki in range(NKT):
                        nc.vector.scalar_tensor_tensor(
                            out=junk_sb, in0=vts[x][:, ds(ki * VCH, VCH)],
                            scalar=0.0, in1=sgb_sb,
                            op0=AL.bypass, op1=AL.mult,
                            accum_out=ctxcols[:, b, ki, c:c + 1])
                if b == 0:
                    ng = len(GROUPS[0])
                    s0 = (gi * NOK * H) // ng
                    s1 = ((gi + 1) * NOK * H) // ng
                    cdma(outw_sb[:, ds(s0, s1 - s0)],
                         ow_p[:, ds(s0, s1 - s0)], gate=group_mms[-1])
                gidx += 1

            if b == 0:
                batch_epilogue(0)

        # query half of the output projection (independent of context)
        op_ps = [p_sc.tile([BPC, 512], f32, tag="sc", name=f"op{x}")
                 for x in range(2)]
        for n in range(2):
            for ki in range(NKT, NOK):
                nc.tensor.matmul(op_ps[n],
                                 blob_sb[:, ds(OFF_QP + (ki - NKT) * BPC, BPC)],
                                 outw_sb[:, ds(ki * H + n * 512, 512)],
                                 start=(ki == NKT), stop=False)

        # keep the PE HAM warm while the batch-1 context backlog drains
        warm2_ps = p_e.tile([128, 128], f32, tag="e", name="warm2")
        for i in range(100):
            nc.tensor.matmul(warm2_ps, warm_sb, warm_sb,
                             start=(i == 0), stop=(i == 99))
        nc.scalar.activation(dbg_sb[:, 4:8], warm2_ps[0:1, 0:4], AF.Copy)
        nc.sync.dma_start(out=dbg_d[:, :], in_=dbg_sb)

        batch_epilogue(1)

        # context half + bias + store
        for n in range(2):
            for ki in range(NKT):
                nc.tensor.matmul(op_ps[n], ctxT_sb[:, ds(ki * BPC, BPC)],
                                 outw_sb[:, ds(ki * H + n * 512, 512)],
                                 start=False, stop=(ki == NKT - 1))
            o_sb = small.tile([BPC, 512], f32, tag="osb", name=f"o{n}")
            nc.vector.tensor_add(o_sb, op_ps[n], outb_sb[:, ds(n * 512, 512)])
            nc.sync.dma_start(out=out_d[:, ds(n * 512, 512)], in_=o_sb)

    nc.compile()
    return nc


def _get_program():
    if "nc" not in _CACHE:
        _CACHE["nc"] = _build_program()
    return _CACHE["nc"]


def _host_prep(query, value, prev_attn, conv_w, conv_b, loc_w, q_w, v_w, bias,
               score_w, score_b, out_w, out_b):
    """Build per-core input maps (layout transforms only)."""
    query = np.asarray(query, np.float32)
    value = np.asarray(value, np.float32)
    prev_attn = np.asarray(prev_attn, np.float32)

    # shifted prev_attn rows + ones row (conv via matmul, bias folded)
    px = np.zeros((B, 4, VL), np.float32)
    px[:, 0, 1:] = prev_attn[:, :-1]
    px[:, 1, :] = prev_attn
    px[:, 2, :-1] = prev_attn[:, 1:]
    px[:, 3, :] = 1.0

    w_aug = np.zeros((4, CO), np.float32)
    w_aug[0:3] = np.asarray(conv_w, np.float32)[:, 0, :].T  # [t, c]
    w_aug[3] = np.asarray(conv_b, np.float32)

    def pack_w(w, nkt):
        # (out_dim, in_dim) weight -> [128, nkt*out_dim] with k-tile-major free
        wt = np.ascontiguousarray(np.asarray(w, np.float32).T)  # (in, out)
        od = wt.shape[1]
        return np.ascontiguousarray(
            wt.reshape(nkt, 128, od).transpose(1, 0, 2).reshape(128, nkt * od))

    locw_pad = np.zeros((128, D), np.float32)
    locw_pad[:CO] = np.asarray(loc_w, np.float32).T
    waug_pad = np.zeros((128, CO), np.float32)
    waug_pad[:4] = w_aug
    score_wR = np.asarray(score_w, np.float32)[0].reshape(NDT, 128).T

    shared = {
        "bias_r": np.ascontiguousarray(
            np.asarray(bias, np.float32).reshape(NDT, 128).T),
        "score_b": np.asarray(score_b, np.float32).reshape(1, 1),
        "ow_p": pack_w(out_w, NOK),
        "out_b2": np.ascontiguousarray(
            np.broadcast_to(np.asarray(out_b, np.float32), (BPC, H))),
    }
    in_maps = []
    for cidx in range(NCORES):
        sl = slice(cidx * BPC, (cidx + 1) * BPC)
        m = dict(shared)
        # value[b, v, h] -> [b, chunk, p, ki*VCH + vv] with h = ki*128 + p,
        # v = chunk*VCH + vv
        vv = value[sl].reshape(BPC, NCHUNK, VCH, NKT, 128)
        m["value_p"] = np.ascontiguousarray(
            vv.transpose(0, 1, 4, 3, 2).reshape(BPC, NCHUNK, 128, NKT * VCH))
        # query[b, 0, h] -> [p, ki*BPC + b]
        qq = query[sl, 0, :].T.reshape(NKT, 128, BPC)
        q_p = np.ascontiguousarray(
            qq.transpose(1, 0, 2).reshape(128, NKT * BPC))
        qwt = np.asarray(q_w, np.float32).T.reshape(NKT, 128, NDT, 128)
        qw_jmaj = qwt.transpose(1, 2, 0, 3).reshape(128, NKT * D)
        m["blob_p"] = np.ascontiguousarray(np.concatenate(
            [pack_w(v_w, NKT), locw_pad, waug_pad, score_wR, q_p,
             qw_jmaj], axis=1))
        m["prevX"] = np.ascontiguousarray(px[sl])
        in_maps.append(m)
    return in_maps


def kernel(query, value, prev_attn, conv_w, conv_b, loc_w, q_w, v_w, bias,
           score_w, score_b, out_w, out_b, seq_len=None, **_unused):
    from concourse.bass_utils import run_bass_kernel_spmd

    nc = _get_program()
    in_maps = _host_prep(query, value, prev_attn, conv_w, conv_b, loc_w,
                         q_w, v_w, bias, score_w, score_b, out_w, out_b)
    res = run_bass_kernel_spmd(nc, in_maps, list(range(NCORES)))
    _CACHE["last_results"] = res
    output = np.zeros((B, 1, H), np.float32)
    attn = np.zeros((B, VL), np.float32)
    for cidx in range(NCORES):
        sl = slice(cidx * BPC, (cidx + 1) * BPC)
        output[sl, 0, :] = res.results[cidx]["out"]
        attn[sl, :] = res.results[cidx]["attn"]
    return output, attn


# revision 29
# speedup vs baseline: 1.2959x; 1.0387x over previous
"""LocationAwareAttention Trainium2 kernel.

Data-parallel over batch: 16 batch elements / 8 cores = 2 per core.
Each core computes, for its 2 batch elements b:
    conv_feat = conv1d(prev_attn) ; lp = conv_feat @ loc_w.T
    qp = query @ q_w.T ; vp = value @ v_w.T
    e  = tanh(qp + vp + lp + bias)              (computed transposed: d on partitions)
    score = e @ score_w.T + score_b ; sg = sigmoid(score)
    attn = sg / sum(sg) ; context = attn @ value
    out = [context | query] @ out_w.T + out_b

Device layout: value is shipped host-transposed/packed (h-major, chunked) so
the dominant matmul (vp) streams it directly as the PE moving operand from
plain contiguous DMAs; the context reduction (contraction over v, which PE
cannot do in this layout) runs on the vector engine as fused
scalar_tensor_tensor ops with accum_out.  All matmul operands are cast
fp32->bf16 inline by the SWDGE DMA engines.

v chunks are processed in pairs so each PE stationary (LDWEIGHTS) serves two
matmuls, and a warm-up matmul burst trips the PE HAM clock gate to 2.4 GHz
before the main stream begins.
"""

import numpy as np
from contextlib import ExitStack

B, VL, H, D, CO = 16, 4096, 1024, 512, 10
NCORES = 8
BPC = B // NCORES          # batches per core = 2
VCH = 512                  # v chunk size
NCHUNK = VL // VCH         # 8
NKT = H // 128             # 8 k-tiles over hidden
NDT = D // 128             # 4 m-tiles over dim
NOK = (2 * H) // 128       # 16 k-tiles over 2*hidden (out proj)

_CACHE = {}


def _build_program():
    import concourse.bass as bass
    import concourse.tile as tile
    from concourse import bacc, mybir
    from concourse.bass import ds
    from concourse.tile_rust import add_dep_helper

    f32 = mybir.dt.float32
    bf16 = mybir.dt.bfloat16
    AF = mybir.ActivationFunctionType
    AL = mybir.AluOpType

    nc = bacc.Bacc(None, target_bir_lowering=False, debug=False,
                   num_devices=NCORES)

    # ---- DRAM I/O (all pre-packed host-side; device DMAs are plain 2D) ---
    value_p = nc.dram_tensor("value_p", [BPC, NCHUNK, 128, NKT * VCH], f32,
                             kind="ExternalInput")
    prevX = nc.dram_tensor("prevX", [BPC, 4, VL], f32, kind="ExternalInput")
    # packed: [vw 4096 | qw 4096 | swT 4 | q_p 16 | locw 512 | waug 10]
    BLOBW = NKT * D * 2 + NDT + NKT * BPC + D + CO
    blob_p = nc.dram_tensor("blob_p", [128, BLOBW], f32, kind="ExternalInput")
    bias_r = nc.dram_tensor("bias_r", [128, NDT], f32, kind="ExternalInput")
    score_b = nc.dram_tensor("score_b", [1, 1], f32, kind="ExternalInput")
    ow_p = nc.dram_tensor("ow_p", [128, NOK * H], f32, kind="ExternalInput")
    out_b2 = nc.dram_tensor("out_b2", [BPC, H], f32, kind="ExternalInput")

    out_d = nc.dram_tensor("out", [BPC, H], f32, kind="ExternalOutput")
    attn_d = nc.dram_tensor("attn", [BPC, VL], f32, kind="ExternalOutput")
    dbg_d = nc.dram_tensor("dbg", [1, 8], f32, kind="ExternalOutput")

    with tile.TileContext(nc) as tc, ExitStack() as ctx:
        singles = ctx.enter_context(tc.tile_pool(name="singles", bufs=1))
        vt_pool = ctx.enter_context(tc.tile_pool(name="vt", bufs=6))
        te_pool = ctx.enter_context(tc.tile_pool(name="te", bufs=6))
        sgb_pool = ctx.enter_context(tc.tile_pool(name="sgb", bufs=2))
        small = ctx.enter_context(tc.tile_pool(name="small", bufs=2))
        p_e = ctx.enter_context(tc.tile_pool(name="p_e", bufs=3, space="PSUM"))
        p_sc = ctx.enter_context(tc.tile_pool(name="p_sc", bufs=2, space="PSUM"))
        p_sgb = ctx.enter_context(tc.tile_pool(name="p_sgb", bufs=1, space="PSUM"))
        p_misc = ctx.enter_context(tc.tile_pool(name="p_misc", bufs=2, space="PSUM"))

        # ---- critical-path loads first, serialized so the earliest-needed
        # transfer gets full DMA bandwidth (concurrent SWDGE queues are
        # drained round-robin, which would finish everything late together)
        def cdma(out, in_, gate=None):
            dd = nc.gpsimd.dma_start(out=out, in_=in_)
            if gate is not None:
                add_dep_helper(dd.ins, gate.ins, reason="dma start gate")
            return dd

        vt_sbs = {}
        blob_sb = singles.tile([128, BLOBW], bf16)
        # views into the packed weights blob: [vw | locw | waug | swT | q_p | qw]
        OFF_VW = 0
        OFF_LW = NKT * D
        OFF_WA = OFF_LW + D
        OFF_SW = OFF_WA + CO
        OFF_QP = OFF_SW + NDT
        OFF_QW = OFF_QP + NKT * BPC
        BLOBA = OFF_QW
        RESTA = BLOBA - NKT * D   # locw/waug/swT/q_p tail of part A
        qsz = NKT * 128

        # Startup-critical tensors ride the two HWDGE rings as plain fp32
        # (full DMA rate; the SWDGE cast path tops out around half rate for
        # a single in-flight transfer) and are cast to bf16 on the
        # still-idle vector engine.
        vt0 = vt_pool.tile([128, NKT * VCH], bf16, tag="vt", name="vt00")
        cdma(vt0, value_p[0, 0])
        cdma(blob_sb[:, ds(0, BLOBA)], blob_p[:, ds(0, BLOBA)])
        px0_sb = small.tile([4, VL], bf16, tag="px", name="px0")
        cdma(px0_sb, prevX[0])
        cdma(blob_sb[:, ds(OFF_QW, qsz)], blob_p[:, ds(OFF_QW, qsz)])
        vw_sb = blob_sb[:, ds(OFF_VW, NKT * D)]
        qw_sb = blob_sb[:, ds(OFF_QW, NKT * D)]
        swT_sb = blob_sb[:, ds(OFF_SW, NDT)]
        locw_sb = blob_sb[0:CO, ds(OFF_LW, D)]
        waug_sb = blob_sb[0:4, ds(OFF_WA, CO)]

        # ---- PE warm-up: dense junk matmuls trip HAM to 2.4 GHz ----------
        warm_sb = singles.tile([128, 128], bf16)
        nc.vector.memset(warm_sb, 0.001)
        warm_ps = p_e.tile([128, 128], f32, tag="e")
        warm_last = None
        for i in range(190):
            warm_last = nc.tensor.matmul(warm_ps, warm_sb, warm_sb,
                                         start=(i == 0), stop=(i == 189))
        dbg_sb = singles.tile([1, 8], f32)
        nc.scalar.activation(dbg_sb[:, 0:4], warm_ps[0:1, 0:4], AF.Copy)

        # second-wave loads start once the warm-up burst retires, leaving
        # the full DMA bandwidth to the group-0 prerequisites before that
        for jq in range(1, NDT):
            cdma(blob_sb[:, ds(OFF_QW + jq * qsz, qsz)],
                 blob_p[:, ds(OFF_QW + jq * qsz, qsz)], gate=warm_last)
        px1_sb = small.tile([4, VL], bf16, tag="px", name="px1")
        cdma(px1_sb, prevX[1], gate=warm_last)
        px_sbs = [px0_sb, px1_sb]

        # ---- remaining resident weights ----------------------------------
        bias_sb = singles.tile([128, NDT], f32)
        nc.sync.dma_start(out=bias_sb, in_=bias_r[:, :])
        sb_sb = singles.tile([1, 1], f32)
        nc.sync.dma_start(out=sb_sb, in_=score_b[:, :])
        outb_sb = singles.tile([BPC, H], f32)
        nc.sync.dma_start(out=outb_sb, in_=out_b2[:, :])

        outw_sb = singles.tile([128, NOK * H], bf16)
        ctxT_sb = singles.tile([128, NKT * BPC], bf16)
        ones_sb = singles.tile([1, 128], bf16)
        nc.vector.memset(ones_sb, 1.0)
        onesf_sb = singles.tile([1, 128], f32)
        nc.vector.memset(onesf_sb, 1.0)

        # per-batch bookkeeping
        sg_sb = singles.tile([1, BPC, VL], f32)
        ssum_sb = singles.tile([1, BPC, NCHUNK], f32)
        ctxcols = singles.tile([128, BPC, NKT, NCHUNK], f32)
        ctxred = singles.tile([128, BPC, NKT], f32)
        recip_sb = singles.tile([1, BPC], f32)
        qpb_sb = singles.tile([128, NDT, BPC], f32)
        junk_sb = singles.tile([128, VCH], bf16)

        # conv-feature chunks are produced inside the main loop (keeps PE
        # dense from the start); qp is emitted after the first group.

        def batch_epilogue(b):
            """context reduce -> combT ctx cols; then attn out.  (ssum
            reduction + reciprocal were already emitted after the batch's
            last sigmoid.)"""
            nc.vector.tensor_reduce(ctxred[:, b, :], ctxcols[:, b, :, :],
                                    axis=mybir.AxisListType.X, op=AL.add)
            # fp32 rank-1 broadcast of 1/S to 128 partitions
            rb_ps = p_sgb.tile([128, 1], f32, tag="sgbp", name=f"rb{b}")
            nc.tensor.matmul(rb_ps, onesf_sb, recip_sb[:, b:b + 1],
                             start=True, stop=True)
            rb_sb = small.tile([128, 1], f32, tag="rbs", name=f"rbs{b}")
            nc.scalar.copy(rb_sb, rb_ps)
            ctxT_v = ctxT_sb.rearrange("p (k b) -> p k b", b=BPC)
            nc.vector.tensor_scalar_mul(ctxT_v[:, 0:NKT, b], ctxred[:, b, :],
                                        rb_sb)
            attn_sb = small.tile([1, VL], f32, tag="attn", name=f"attn{b}",
                                 bufs=1)
            nc.vector.tensor_scalar_mul(attn_sb, sg_sb[:, b, :],
                                        recip_sb[:, b:b + 1])
            nc.sync.dma_start(out=attn_d[b], in_=attn_sb)

        # ---- main loop: chunk groups, shared stationaries ---------------
        GROUPS = {0: [(0,), (1,), (2,), (3, 4), (5, 6), (7,)],
                  1: [(0, 1), (2, 3), (4, 5), (6,), (7,)]}
        def emit_qp(j):
            qpp = p_misc.tile([128, BPC], f32, tag="misc", name=f"qpp{j}")
            for ki in range(NKT):
                nc.tensor.matmul(
                    qpp, qw_sb[:, ds(j * NKT * 128 + ki * 128, 128)],
                    blob_sb[:, ds(OFF_QP + ki * BPC, BPC)],
                    start=(ki == 0), stop=(ki == NKT - 1))
            nc.vector.tensor_scalar_add(qpb_sb[:, j, :], qpp,
                                        bias_sb[:, j:j + 1])
        cf_sbs = []
        group_mms = []
        gidx = 0
        for b in range(BPC):
            cf_sb = small.tile([CO, VL], bf16, tag="cf", name=f"cf{b}")
            cf_sbs.append(cf_sb)
            for gi, grp in enumerate(GROUPS[b]):
                L = len(grp)
                vts = []
                gate = group_mms[gidx - 2] if gidx >= 2 else warm_last
                for c in grp:
                    if (b, c) in vt_sbs:
                        vts.append(vt_sbs[(b, c)])
                    else:
                        vt = vt_pool.tile([128, NKT * VCH], bf16, tag="vt",
                                          name=f"vt{b}_{c}")
                        cdma(vt, value_p[b, c], gate=gate)
                        vts.append(vt)
                # conv features for this group's chunks
                for c in grp:
                    cfp = p_misc.tile([CO, VCH], f32, tag="misc",
                                      name=f"cfp{b}_{c}")
                    nc.tensor.matmul(cfp, waug_sb, px_sbs[b][:, ds(c * VCH, VCH)],
                                     start=True, stop=True)
                    nc.scalar.activation(cf_sb[:, ds(c * VCH, VCH)], cfp,
                                         AF.Copy)
                sc_ps = [p_sc.tile([1, VCH], f32, tag="sc", name=f"sc{b}_{gi}_{x}")
                         for x in range(L)]
                tes = {}
                for j in range(NDT):
                    e_ps = [p_e.tile([128, VCH], f32, tag="e",
                                     name=f"e{b}_{gi}_{j}_{x}")
                            for x in range(L)]
                    for ki in range(NKT):
                        lhs = vw_sb[:, ds(ki * D + j * 128, 128)]
                        for x in range(L):
                            mm = nc.tensor.matmul(e_ps[x], lhs,
                                                  vts[x][:, ds(ki * VCH, VCH)],
                                                  start=(ki == 0), stop=False)
                            if j == 0 and ki == 0 and x == 0:
                                group_mms.append(mm)
                    lhs = locw_sb[:, ds(j * 128, 128)]
                    for x in range(L):
                        nc.tensor.matmul(e_ps[x], lhs,
                                         cf_sb[:, ds(grp[x] * VCH, VCH)],
                                         start=False, stop=True)
                    if b == 0 and gi == 0:
                        # query projection for this j slots in here: its
                        # j-slice of qw lands while the e-block streams
                        emit_qp(j)
                    for x in range(L):
                        t = te_pool.tile([128, VCH], bf16, tag="te",
                                         name=f"te{b}_{gi}_{j}_{x}")
                        nc.scalar.activation(t, e_ps[x], AF.Tanh,
                                             bias=qpb_sb[:, j, b:b + 1])
                        tes[(j, x)] = t
                    if j > 0:
                        lhs = swT_sb[:, j - 1:j]
                        for x in range(L):
                            nc.tensor.matmul(sc_ps[x], lhs, tes[(j - 1, x)],
                                             start=(j == 1), stop=False)
                jl = NDT - 1
                lhs = swT_sb[:, jl:jl + 1]
                for x in range(L):
                    nc.tensor.matmul(sc_ps[x], lhs, tes[(jl, x)],
                                     start=False, stop=True)
                for x, c in enumerate(grp):
                    # sigmoid + per-chunk sum of sg
                    nc.scalar.activation(sg_sb[:, b, ds(c * VCH, VCH)],
                                         sc_ps[x], AF.Sigmoid,
                                         bias=sb_sb[:, 0:1],
                                         accum_out=ssum_sb[:, b, c:c + 1])
                    if c == NCHUNK - 1:
                        # 1/S ready before the context backlog drains
                        ssr = small.tile([1, 1], f32, tag="ssr",
                                         name=f"ssr{b}")
                        nc.vector.tensor_reduce(ssr, ssum_sb[:, b, :],
                                                axis=mybir.AxisListType.X,
                                                op=AL.add)
                        nc.vector.reciprocal(recip_sb[:, b:b + 1], ssr)
                    # bf16 copy of sg for the broadcast matmul
                    sgc_sb = sgb_pool.tile([1, VCH], bf16, tag="sgc",
                                           name=f"sgc{b}_{c}")
                    nc.scalar.activation(sgc_sb, sg_sb[:, b, ds(c * VCH, VCH)],
                                         AF.Copy)
                    sgb_ps = p_sgb.tile([128, VCH], f32, tag="sgbp",
                                        name=f"sgbp{b}_{c}")
                    nc.tensor.matmul(sgb_ps, ones_sb, sgc_sb,
                                     start=True, stop=True)
                    sgb_sb = sgb_pool.tile([128, VCH], bf16, tag="sgb",
                                           name=f"sgb{b}_{c}")
                    nc.scalar.activation(sgb_sb, sgb_ps, AF.Copy)
                    # context partials: ctxcols[., b, ki, c] = sum_v vt*sg
                    for ki in range(NKT):
                        nc.vector.scalar_tensor_tensor(
                            out=junk_sb, in0=vts[x][:, ds(ki * VCH, VCH)],
                            scalar=0.0, in1=sgb_sb,
                            op0=AL.bypass, op1=AL.mult,
                            accum_out=ctxcols[:, b, ki, c:c + 1])
                if b == 0:
                    ng = len(GROUPS[0])
                    s0 = (gi * NOK * H) // ng
                    s1 = ((gi + 1) * NOK * H) // ng
                    cdma(outw_sb[:, ds(s0, s1 - s0)],
                         ow_p[:, ds(s0, s1 - s0)], gate=group_mms[-1])
                gidx += 1

            if b == 0:
                batch_epilogue(0)

        # query half of the output projection (independent of context)
        op_ps = [p_sc.tile([BPC, 512], f32, tag="sc", name=f"op{x}")
                 for x in range(2)]
        for n in range(2):
            for ki in range(NKT, NOK):
                nc.tensor.matmul(op_ps[n],
                                 blob_sb[:, ds(OFF_QP + (ki - NKT) * BPC, BPC)],
                                 outw_sb[:, ds(ki * H + n * 512, 512)],
                                 start=(ki == NKT), stop=False)

        # keep the PE HAM warm while the batch-1 context backlog drains
        warm2_ps = p_e.tile([128, 128], f32, tag="e", name="warm2")
        for i in range(100):
            nc.tensor.matmul(warm2_ps, warm_sb, warm_sb,
                             start=(i == 0), stop=(i == 99))
        nc.scalar.activation(dbg_sb[:, 4:8], warm2_ps[0:1, 0:4], AF.Copy)
        nc.sync.dma_start(out=dbg_d[:, :], in_=dbg_sb)

        batch_epilogue(1)

        # context half + bias + store
        for n in range(2):
            for ki in range(NKT):
                nc.tensor.matmul(op_ps[n], ctxT_sb[:, ds(ki * BPC, BPC)],
                                 outw_sb[:, ds(ki * H + n * 512, 512)],
                                 start=False, stop=(ki == NKT - 1))
            o_sb = small.tile([BPC, 512], f32, tag="osb", name=f"o{n}")
            nc.vector.tensor_add(o_sb, op_ps[n], outb_sb[:, ds(n * 512, 512)])
            nc.sync.dma_start(out=out_d[:, ds(n * 512, 512)], in_=o_sb)

    nc.compile()
    return nc


def _get_program():
    if "nc" not in _CACHE:
        _CACHE["nc"] = _build_program()
    return _CACHE["nc"]


def _host_prep(query, value, prev_attn, conv_w, conv_b, loc_w, q_w, v_w, bias,
               score_w, score_b, out_w, out_b):
    """Build per-core input maps (layout transforms only)."""
    query = np.asarray(query, np.float32)
    value = np.asarray(value, np.float32)
    prev_attn = np.asarray(prev_attn, np.float32)

    # shifted prev_attn rows + ones row (conv via matmul, bias folded)
    px = np.zeros((B, 4, VL), np.float32)
    px[:, 0, 1:] = prev_attn[:, :-1]
    px[:, 1, :] = prev_attn
    px[:, 2, :-1] = prev_attn[:, 1:]
    px[:, 3, :] = 1.0

    w_aug = np.zeros((4, CO), np.float32)
    w_aug[0:3] = np.asarray(conv_w, np.float32)[:, 0, :].T  # [t, c]
    w_aug[3] = np.asarray(conv_b, np.float32)

    def pack_w(w, nkt):
        # (out_dim, in_dim) weight -> [128, nkt*out_dim] with k-tile-major free
        wt = np.ascontiguousarray(np.asarray(w, np.float32).T)  # (in, out)
        od = wt.shape[1]
        return np.ascontiguousarray(
            wt.reshape(nkt, 128, od).transpose(1, 0, 2).reshape(128, nkt * od))

    locw_pad = np.zeros((128, D), np.float32)
    locw_pad[:CO] = np.asarray(loc_w, np.float32).T
    waug_pad = np.zeros((128, CO), np.float32)
    waug_pad[:4] = w_aug
    score_wR = np.asarray(score_w, np.float32)[0].reshape(NDT, 128).T

    shared = {
        "bias_r": np.ascontiguousarray(
            np.asarray(bias, np.float32).reshape(NDT, 128).T),
        "score_b": np.asarray(score_b, np.float32).reshape(1, 1),
        "ow_p": pack_w(out_w, NOK),
        "out_b2": np.ascontiguousarray(
            np.broadcast_to(np.asarray(out_b, np.float32), (BPC, H))),
    }
    in_maps = []
    for cidx in range(NCORES):
        sl = slice(cidx * BPC, (cidx + 1) * BPC)
        m = dict(shared)
        # value[b, v, h] -> [b, chunk, p, ki*VCH + vv] with h = ki*128 + p,
        # v = chunk*VCH + vv
        vv = value[sl].reshape(BPC, NCHUNK, VCH, NKT, 128)
        m["value_p"] = np.ascontiguousarray(
            vv.transpose(0, 1, 4, 3, 2).reshape(BPC, NCHUNK, 128, NKT * VCH))
        # query[b, 0, h] -> [p, ki*BPC + b]
        qq = query[sl, 0, :].T.reshape(NKT, 128, BPC)
        q_p = np.ascontiguousarray(
            qq.transpose(1, 0, 2).reshape(128, NKT * BPC))
        qwt = np.asarray(q_w, np.float32).T.reshape(NKT, 128, NDT, 128)
        qw_jmaj = qwt.transpose(1, 2, 0, 3).reshape(128, NKT * D)
        m["blob_p"] = np.ascontiguousarray(np.concatenate(
            [pack_w(v_w, NKT), locw_pad, waug_pad, score_wR, q_p,
             qw_jmaj], axis=1))
        m["prevX"] = np.ascontiguousarray(px[sl])
        in_maps.append(m)
    return in_maps


def kernel(query, value, prev_attn, conv_w, conv_b, loc_w, q_w, v_w, bias,
           score_w, score_b, out_w, out_b, seq_len=None, **_unused):
    from concourse.bass_utils import run_bass_kernel_spmd

    nc = _get_program()
    in_maps = _host_prep(query, value, prev_attn, conv_w, conv_b, loc_w,
                         q_w, v_w, bias, score_w, score_b, out_w, out_b)
    res = run_bass_kernel_spmd(nc, in_maps, list(range(NCORES)))
    _CACHE["last_results"] = res
    output = np.zeros((B, 1, H), np.float32)
    attn = np.zeros((B, VL), np.float32)
    for cidx in range(NCORES):
        sl = slice(cidx * BPC, (cidx + 1) * BPC)
        output[sl, 0, :] = res.results[cidx]["out"]
        attn[sl, :] = res.results[cidx]["attn"]
    return output, attn
